# revision 8
# baseline (speedup 1.0000x reference)
"""GNN message-passing kernel for 8 Trainium2 NeuronCores.

Strategy: 1-D node partition (6250 nodes/core). Each layer:
  - node features h kept in transposed SBUF layout hT [128 latent, nodes]
  - h rows are PE-transposed out and AllGathered into a full [50000, 128]
    DRAM table per layer
  - edges are owned by the destination core, sorted by (dest block, src half);
    h[fro] is fetched with dma_gather (int16 idx -> table split in two halves)
  - segment-sum via PE matmul: msgT_block += M_tile^T @ S_tile where S is the
    per-tile one-hot dest selection matrix built on DVE (iota == dest)
  - combine: hT_next = relu(W1^T hT + W2^T (msgT * invdeg) + b)
"""

import os
import numpy as np

N_CORES = 8
P = 128
CHUNK_TILES = 8          # tiles per dma_gather instruction (<=1024 idx, ring cap)
NQ = 4                   # SWDGE queues for gather round-robin
SPLIT = 32768            # int16-addressable table half boundary


def _host_prep(nodes_feats, adj_list, W_in, b_in, W_comb, b_comb, W_out, b_out):
    N, IN_DIM = nodes_feats.shape
    E = adj_list.shape[0]
    LATENT = W_in.shape[1]
    OUT_DIM = W_out.shape[1]
    NLAYER = W_comb.shape[0]
    NPC = N // N_CORES
    NBLK = (NPC + P - 1) // P

    fro = np.asarray(adj_list[:, 0], dtype=np.int64)
    to = np.asarray(adj_list[:, 1], dtype=np.int64)
    deg = np.bincount(to, minlength=N).astype(np.float32)
    invd = 1.0 / np.maximum(deg, 1.0)

    XT = np.ascontiguousarray(np.asarray(nodes_feats, dtype=np.float32).T)  # [IN, N]

    # per (core, block, half) edge lists
    per = [[[None, None] for _ in range(NBLK)] for _ in range(N_CORES)]
    counts = np.zeros((N_CORES, NBLK, 2), dtype=np.int64)
    for c in range(N_CORES):
        sel = (to >= c * NPC) & (to < (c + 1) * NPC)
        f_c = fro[sel]
        d_c = to[sel] - c * NPC
        blk = d_c // P
        half = (f_c >= SPLIT).astype(np.int64)
        order = np.lexsort((half, blk))
        f_c, d_c, blk, half = f_c[order], d_c[order], blk[order], half[order]
        # boundaries per (blk, half)
        key = blk * 2 + half
        for k in np.unique(key):
            m = key == k
            b, h = int(k) // 2, int(k) % 2
            per[c][b][h] = (f_c[m], d_c[m] - b * P)
            counts[c, b, h] = int(m.sum())

    # static tiles-per-(block,half): max over cores
    tp = np.zeros((NBLK, 2), dtype=np.int64)
    for b in range(NBLK):
        for h in range(2):
            mx = counts[:, b, h].max()
            tp[b, h] = -(-mx // P)  # ceil; 0 if no core has edges here

    # static tile list + chunk list
    tiles = []          # (block, half, first_of_block, last_of_block)
    for b in range(NBLK):
        nt_a, nt_b = int(tp[b, 0]), int(tp[b, 1])
        ntot = nt_a + nt_b
        for j in range(nt_a):
            tiles.append((b, 0, j == 0, j == ntot - 1))
        for j in range(nt_b):
            tiles.append((b, 1, nt_a + j == 0, nt_a + j == ntot - 1))
    T_total = len(tiles)

    # chunks: maximal same-half runs, split to <= CHUNK_TILES
    chunks = []         # (t0, ntiles, half)
    t = 0
    while t < T_total:
        h = tiles[t][1]
        r = t
        while r < T_total and tiles[r][1] == h and r - t < CHUNK_TILES:
            r += 1
        chunks.append((t, r - t, h))
        t = r

    # per-core slot arrays
    gat_idx = np.zeros((N_CORES, T_total, P), dtype=np.int16)
    dest_loc = np.full((N_CORES, T_total, P), -1.0, dtype=np.float32)
    for c in range(N_CORES):
        ti = 0
        for b in range(NBLK):
            for h in range(2):
                nt = int(tp[b, h])
                if nt == 0:
                    continue
                lst = per[c][b][h]
                if lst is not None:
                    f_l, d_l = lst
                    n = len(f_l)
                    fpad = np.zeros(nt * P, dtype=np.int64)
                    dpad = np.full(nt * P, -1.0, dtype=np.float32)
                    fpad[:n] = f_l - (SPLIT if h else 0)
                    dpad[:n] = d_l
                else:
                    fpad = np.zeros(nt * P, dtype=np.int64)
                    dpad = np.full(nt * P, -1.0, dtype=np.float32)
                gat_idx[c, ti:ti + nt] = fpad.reshape(nt, P).astype(np.int16)
                dest_loc[c, ti:ti + nt] = dpad.reshape(nt, P)
                ti += nt
        assert ti == T_total

    # snake-16 idx layout per chunk, concatenated along free dim
    idx16 = np.zeros((N_CORES, P, T_total * (P // 16)), dtype=np.int16)
    for c in range(N_CORES):
        col = 0
        for (t0, nt, h) in chunks:
            nidx = nt * P
            flat = gat_idx[c, t0:t0 + nt].reshape(nidx)   # logical i = (t-t0)*128+p
            w = flat.reshape(nidx // 16, 16).T            # [16, nidx//16]
            idx16[c, :, col:col + nidx // 16] = np.tile(w, (8, 1))
            col += nidx // 16
        assert col == T_total * (P // 16)

    dest_all = np.ascontiguousarray(dest_loc.transpose(0, 2, 1))  # [C, 128, T]

    invd_b = np.empty((N_CORES, P, NPC), dtype=np.float32)
    for c in range(N_CORES):
        invd_b[c] = np.broadcast_to(invd[c * NPC:(c + 1) * NPC], (P, NPC))

    Wc = np.asarray(W_comb, dtype=np.float32)
    meta = dict(N=N, E=E, IN=IN_DIM, L=LATENT, OUT=OUT_DIM, NLAYER=NLAYER,
                NPC=NPC, NBLK=NBLK, tiles=tiles, chunks=chunks, T_total=T_total)
    host = dict(
        XT=XT, idx16=idx16, dest_all=dest_all, invd_b=invd_b,
        W_in=np.asarray(W_in, np.float32),
        b_in=np.asarray(b_in, np.float32).reshape(LATENT, 1),
        Wc=Wc,
        bc=np.asarray(b_comb, np.float32).reshape(NLAYER, LATENT, 1),
        W_out=np.asarray(W_out, np.float32),
        b_out_b=np.broadcast_to(np.asarray(b_out, np.float32), (P, OUT_DIM)).copy(),
    )
    return meta, host


def _build_program(meta, timing_reps=1):
    import concourse.bass as bass
    import concourse.bacc as bacc
    import concourse.tile as tile
    import concourse.mybir as mybir
    from concourse.masks import make_identity

    N, IN_DIM, L, OUT_DIM = meta["N"], meta["IN"], meta["L"], meta["OUT"]
    NLAYER, NPC, NBLK = meta["NLAYER"], meta["NPC"], meta["NBLK"]
    tiles, chunks, T_total = meta["tiles"], meta["chunks"], meta["T_total"]
    IW = T_total * (P // 16)
    f32 = mybir.dt.float32

    nc = bacc.Bacc("TRN2", target_bir_lowering=False, debug=False,
                   num_devices=N_CORES, num_swdge_queues=NQ)

    xt_in = nc.dram_tensor("xt", [IN_DIM, NPC], f32, kind="ExternalInput").ap()
    idx_in = nc.dram_tensor("idx16", [P, IW], mybir.dt.int16, kind="ExternalInput").ap()
    dest_in = nc.dram_tensor("dest", [P, T_total], f32, kind="ExternalInput").ap()
    invd_in = nc.dram_tensor("invd", [P, NPC], f32, kind="ExternalInput").ap()
    win_in = nc.dram_tensor("w_in", [IN_DIM, L], f32, kind="ExternalInput").ap()
    bin_in = nc.dram_tensor("b_in", [L, 1], f32, kind="ExternalInput").ap()
    wc_in = nc.dram_tensor("w_comb", [NLAYER, 2 * L, L], f32, kind="ExternalInput").ap()
    bc_in = nc.dram_tensor("b_comb", [NLAYER, L, 1], f32, kind="ExternalInput").ap()
    wout_in = nc.dram_tensor("w_out", [L, OUT_DIM], f32, kind="ExternalInput").ap()
    bout_in = nc.dram_tensor("b_out", [P, OUT_DIM], f32, kind="ExternalInput").ap()
    tok_in = nc.dram_tensor("tok", [P, 1], f32, kind="ExternalInput").ap()
    out_t = nc.dram_tensor("out", [NPC, OUT_DIM], f32, kind="ExternalOutput").ap()
    tok_out = nc.dram_tensor("tok_out", [P, 1], f32, kind="ExternalOutput").ap()

    NT_NODE = (NPC + P - 1) // P          # node tiles (49)
    CCH = 512                              # combine chunk width
    NCH = (NPC + CCH - 1) // CCH

    with tile.TileContext(nc) as tc:
        with (
            tc.tile_pool(name="const", bufs=1) as cpool,
            tc.tile_pool(name="hbuf", bufs=1) as hpool,
            tc.tile_pool(name="msg", bufs=1) as mpool,
            tc.tile_pool(name="xt", bufs=3) as xtpool,
            tc.tile_pool(name="g", bufs=4) as gpool,
            tc.tile_pool(name="s", bufs=4) as spool,
            tc.tile_pool(name="rows", bufs=3) as rpool,
            tc.tile_pool(name="osb", bufs=3) as opool,
            tc.tile_pool(name="pm", bufs=2, space="PSUM") as pm,       # msg blocks
            tc.tile_pool(name="pc", bufs=2, space="PSUM") as pc,       # combine
            tc.tile_pool(name="pt", bufs=2, space="PSUM") as pt,       # transpose
            tc.tile_pool(name="ph", bufs=2, space="PSUM") as ph,       # head
            tc.tile_pool(name="dram", bufs=1, space="DRAM") as dram,
        ):
            # ---- constants / metadata
            ident = cpool.tile([P, P], f32)
            make_identity(nc, ident[:])
            iota_i = cpool.tile([P, P], mybir.dt.int32)
            nc.gpsimd.iota(iota_i[:], pattern=[[1, P]], base=0, channel_multiplier=0)
            iota_f = cpool.tile([P, P], f32)
            nc.vector.tensor_copy(iota_f[:], iota_i[:])

            idx16_t = cpool.tile([P, IW], mybir.dt.int16)
            nc.sync.dma_start(out=idx16_t[:], in_=idx_in[:])
            dest_t = cpool.tile([P, T_total], f32)
            nc.sync.dma_start(out=dest_t[:], in_=dest_in[:])
            invd_t = cpool.tile([P, NPC], f32)
            nc.sync.dma_start(out=invd_t[:], in_=invd_in[:])
            win_t = cpool.tile([IN_DIM, L], f32)
            nc.sync.dma_start(out=win_t[:], in_=win_in[:])
            bin_t = cpool.tile([L, 1], f32)
            nc.sync.dma_start(out=bin_t[:], in_=bin_in[:])
            wc_t = [[cpool.tile([L, L], f32, tag=f"wc{l}{k}", name=f"wc{l}{k}")
                     for k in range(2)] for l in range(NLAYER)]
            bc_t = [cpool.tile([L, 1], f32, tag=f"bc{l}", name=f"bc{l}")
                    for l in range(NLAYER)]
            for l in range(NLAYER):
                nc.sync.dma_start(out=wc_t[l][0][:], in_=wc_in[l, 0:L, :])
                nc.sync.dma_start(out=wc_t[l][1][:], in_=wc_in[l, L:2 * L, :])
                nc.sync.dma_start(out=bc_t[l][:], in_=bc_in[l])
            wout_t = cpool.tile([L, OUT_DIM], f32)
            nc.sync.dma_start(out=wout_t[:], in_=wout_in[:])
            bout_t = cpool.tile([P, OUT_DIM], f32)
            nc.sync.dma_start(out=bout_t[:], in_=bout_in[:])
            tokt = cpool.tile([P, 1], f32, name="tokt")
            nc.sync.dma_start(out=tokt[:], in_=tok_in[:])
            nc.sync.dma_start(out=tok_out[:], in_=tokt[:])

            def rows_and_bounce(hT, bounce):
                """PE-transpose hT [L, NPC] tiles into row layout, DMA to bounce."""
                for ntile in range(NT_NODE):
                    w = min(P, NPC - ntile * P)
                    sl = slice(ntile * P, ntile * P + w)
                    tps = pt.tile([P, P], f32, tag="tp")
                    nc.tensor.transpose(out=tps[:w, :], in_=hT[:, sl], identity=ident[:])
                    rows = rpool.tile([P, P], f32, tag="rows")
                    nc.vector.tensor_copy(rows[:w, :], tps[:w, :])
                    nc.sync.dma_start(out=bounce[sl, :], in_=rows[:w, :])

            def layer_msg_combine(li, hT_prev, hT_next, tab, qoff):
                msgT = mpool.tile([P, NPC], f32, tag="msgT")
                psum_blk = {}
                for ci, (t0, nt, half) in enumerate(chunks):
                    g = gpool.tile([P, CHUNK_TILES * P], f32, tag="g")
                    src = tab[0:SPLIT, :] if half == 0 else tab[SPLIT:N, :]
                    nidx = nt * P
                    nc.gpsimd.dma_gather(
                        out_ap=g[:, 0:nt * P].rearrange("p (t d) -> p t d", d=P),
                        in_ap=src,
                        idxs_ap=idx16_t[:, t0 * (P // 16):(t0 + nt) * (P // 16)],
                        num_idxs=nidx, num_idxs_reg=nidx, elem_size=P,
                        queue_num=(qoff + ci) % NQ,
                    )
                    S = spool.tile([P, CHUNK_TILES * P], f32, tag="s")
                    for j in range(nt):
                        t = t0 + j
                        b, _, first, last = tiles[t]
                        nc.vector.tensor_scalar(
                            out=S[:, j * P:(j + 1) * P], in0=iota_f[:],
                            scalar1=dest_t[:, t:t + 1], scalar2=None,
                            op0=mybir.AluOpType.is_equal,
                        )
                        if first:
                            psum_blk[b] = pm.tile([P, P], f32, tag="pmblk",
                                                  name=f"pmblk{li}_{b}")
                        nc.tensor.matmul(
                            out=psum_blk[b][:],
                            lhsT=g[:, j * P:(j + 1) * P],
                            rhs=S[:, j * P:(j + 1) * P],
                            start=first, stop=last,
                        )
                        if last:
                            wb = min(P, NPC - b * P)
                            bsl = slice(b * P, b * P + wb)
                            nc.vector.tensor_tensor(
                                out=msgT[:, bsl], in0=psum_blk[b][:, :wb],
                                in1=invd_t[:, bsl], op=mybir.AluOpType.mult,
                            )
                            del psum_blk[b]
                # blocks with no tiles at all: zero msg
                covered = {b for (b, _, _, _) in tiles}
                for b in range(NBLK):
                    if b not in covered:
                        wb = min(P, NPC - b * P)
                        nc.vector.memset(msgT[:, b * P:b * P + wb], 0.0)
                # combine
                for j in range(NCH):
                    w = min(CCH, NPC - j * CCH)
                    sl = slice(j * CCH, j * CCH + w)
                    psum = pc.tile([P, CCH], f32, tag="pcomb")
                    nc.tensor.matmul(out=psum[:, :w], lhsT=wc_t[li][0][:],
                                     rhs=hT_prev[:, sl], start=True, stop=False)
                    nc.tensor.matmul(out=psum[:, :w], lhsT=wc_t[li][1][:],
                                     rhs=msgT[:, sl], start=False, stop=True)
                    nc.scalar.activation(hT_next[:, sl], psum[:, :w],
                                         mybir.ActivationFunctionType.Relu,
                                         bias=bc_t[li][:])

            for rep in range(timing_reps):
                qoff = 0
                # ---- h0 = relu(W_in^T @ X^T + b_in), transposed layout
                hT = [hpool.tile([P, NPC], f32, tag=f"hT{l % 2}", name=f"hT{l}_{rep}")
                      for l in range(NLAYER + 1)]
                for j in range(NCH):
                    w = min(CCH, NPC - j * CCH)
                    sl = slice(j * CCH, j * CCH + w)
                    xt_c = xtpool.tile([IN_DIM, CCH], f32, tag="xt")
                    nc.sync.dma_start(out=xt_c[:, :w], in_=xt_in[:, sl])
                    psum = pc.tile([P, CCH], f32, tag="pcomb")
                    nc.tensor.matmul(out=psum[:, :w], lhsT=win_t[:],
                                     rhs=xt_c[:, :w], start=True, stop=True)
                    nc.scalar.activation(hT[0][:, sl], psum[:, :w],
                                         mybir.ActivationFunctionType.Relu,
                                         bias=bin_t[:])

                # ---- per layer: publish rows, allgather, message+combine
                for li in range(NLAYER):
                    bounce = dram.tile([NPC, P], f32, tag=f"bounce{li}_{rep}")
                    tab = dram.tile([N, P], f32, tag=f"tab{li}_{rep}",
                                    addr_space="Shared")
                    rows_and_bounce(hT[li][:], bounce)
                    nc.gpsimd.collective_compute(
                        "AllGather", mybir.AluOpType.bypass,
                        ins=[bounce[:]], outs=[tab[:]],
                        replica_groups=[list(range(N_CORES))],
                    )
                    layer_msg_combine(li, hT[li][:], hT[li + 1][:], tab, qoff)
                    qoff += len(chunks)

                # ---- output head: out rows = hT_last^T @ W_out + b_out
                for ntile in range(NT_NODE):
                    w = min(P, NPC - ntile * P)
                    sl = slice(ntile * P, ntile * P + w)
                    psum = ph.tile([P, OUT_DIM], f32, tag="phead")
                    nc.tensor.matmul(out=psum[:w, :], lhsT=hT[NLAYER][:, sl],
                                     rhs=wout_t[:], start=True, stop=True)
                    o = opool.tile([P, OUT_DIM], f32, tag="o")
                    nc.vector.tensor_tensor(out=o[:w, :], in0=psum[:w, :],
                                            in1=bout_t[:w, :],
                                            op=mybir.AluOpType.add)
                    nc.sync.dma_start(out=out_t[sl, :], in_=o[:w, :])

    nc.compile()
    return nc


def _in_maps(meta, host):
    maps = []
    NPC = meta["NPC"]
    for c in range(N_CORES):
        maps.append({
            "xt": np.ascontiguousarray(host["XT"][:, c * NPC:(c + 1) * NPC]),
            "idx16": host["idx16"][c],
            "dest": host["dest_all"][c],
            "invd": host["invd_b"][c],
            "w_in": host["W_in"],
            "b_in": host["b_in"],
            "w_comb": host["Wc"],
            "b_comb": host["bc"],
            "w_out": host["W_out"],
            "b_out": host["b_out_b"],
            "tok": np.zeros((P, 1), np.float32),
        })
    return maps


def kernel(nodes_feats, adj_list, W_in, b_in, W_comb, b_comb, W_out, b_out):
    from concourse.bass_utils import run_bass_kernel_spmd

    meta, host = _host_prep(nodes_feats, adj_list, W_in, b_in, W_comb, b_comb,
                            W_out, b_out)
    nc = _build_program(meta)
    res = run_bass_kernel_spmd(nc, _in_maps(meta, host),
                               core_ids=list(range(N_CORES)))
    out = np.concatenate([res.results[c]["out"] for c in range(N_CORES)], axis=0)
    return out.astype(np.float32)


# revision 15
# speedup vs baseline: 1.2064x; 1.2064x over previous
"""GNN message-passing kernel for 8 Trainium2 NeuronCores.

Strategy: 1-D node partition (6250 nodes/core). Each layer:
  - node features h kept in transposed SBUF layout hT [128 latent, nodes]
  - h rows are PE-transposed out and AllGathered into a full [50000, 128]
    DRAM table per layer
  - edges are owned by the destination core, sorted by (dest block, src half);
    h[fro] is fetched with dma_gather (int16 idx -> table split in two halves)
  - segment-sum via PE matmul: msgT_block += M_tile^T @ S_tile where S is the
    per-tile one-hot dest selection matrix built on DVE (iota == dest)
  - combine: hT_next = relu(W1^T hT + W2^T (msgT * invdeg) + b)
"""

import os
import numpy as np

N_CORES = 8
P = 128
CHUNK_TILES = 8          # tiles per dma_gather instruction (<=1024 idx, ring cap)
NQ = 4                   # SWDGE queues for gather round-robin
SPLIT = 32768            # int16-addressable table half boundary


def _host_prep(nodes_feats, adj_list, W_in, b_in, W_comb, b_comb, W_out, b_out):
    N, IN_DIM = nodes_feats.shape
    E = adj_list.shape[0]
    LATENT = W_in.shape[1]
    OUT_DIM = W_out.shape[1]
    NLAYER = W_comb.shape[0]
    NPC = N // N_CORES
    NBLK = (NPC + P - 1) // P

    fro = np.asarray(adj_list[:, 0], dtype=np.int64)
    to = np.asarray(adj_list[:, 1], dtype=np.int64)
    deg = np.bincount(to, minlength=N).astype(np.float32)
    invd = 1.0 / np.maximum(deg, 1.0)

    XT = np.ascontiguousarray(np.asarray(nodes_feats, dtype=np.float32).T)  # [IN, N]

    # per (core, block, half) edge lists
    per = [[[None, None] for _ in range(NBLK)] for _ in range(N_CORES)]
    counts = np.zeros((N_CORES, NBLK, 2), dtype=np.int64)
    for c in range(N_CORES):
        sel = (to >= c * NPC) & (to < (c + 1) * NPC)
        f_c = fro[sel]
        d_c = to[sel] - c * NPC
        blk = d_c // P
        half = (f_c >= SPLIT).astype(np.int64)
        order = np.lexsort((half, blk))
        f_c, d_c, blk, half = f_c[order], d_c[order], blk[order], half[order]
        # boundaries per (blk, half)
        key = blk * 2 + half
        for k in np.unique(key):
            m = key == k
            b, h = int(k) // 2, int(k) % 2
            per[c][b][h] = (f_c[m], d_c[m] - b * P)
            counts[c, b, h] = int(m.sum())

    # static tiles-per-(block,half): max over cores
    tp = np.zeros((NBLK, 2), dtype=np.int64)
    for b in range(NBLK):
        for h in range(2):
            mx = counts[:, b, h].max()
            tp[b, h] = -(-mx // P)  # ceil; 0 if no core has edges here

    # static tile list: phase A (all A-tiles, blocks ascending), then phase B.
    # Each (phase, block) run is a separate PSUM accumulation group; phase-B
    # copyout adds into msgT (phase-A copies).
    tiles = []          # (block, half, first_of_run, last_of_run)
    for h in range(2):
        for b in range(NBLK):
            nt = int(tp[b, h])
            for j in range(nt):
                tiles.append((b, h, j == 0, j == nt - 1))
    T_total = len(tiles)
    has_half = [[tp[b, 0] > 0, tp[b, 1] > 0] for b in range(NBLK)]

    # chunks: same-half runs (span blocks), split to <= CHUNK_TILES
    chunks = []         # (t0, ntiles, half)
    t = 0
    while t < T_total:
        h = tiles[t][1]
        r = t
        while r < T_total and tiles[r][1] == h and r - t < CHUNK_TILES:
            r += 1
        chunks.append((t, r - t, h))
        t = r

    # per-core slot arrays
    gat_idx = np.zeros((N_CORES, T_total, P), dtype=np.int16)
    dest_loc = np.full((N_CORES, T_total, P), -1.0, dtype=np.float32)
    for c in range(N_CORES):
        ti = 0
        for h in range(2):
            for b in range(NBLK):
                nt = int(tp[b, h])
                if nt == 0:
                    continue
                lst = per[c][b][h]
                if lst is not None:
                    f_l, d_l = lst
                    n = len(f_l)
                    fpad = np.zeros(nt * P, dtype=np.int64)
                    dpad = np.full(nt * P, -1.0, dtype=np.float32)
                    fpad[:n] = f_l - (SPLIT if h else 0)
                    dpad[:n] = d_l
                else:
                    fpad = np.zeros(nt * P, dtype=np.int64)
                    dpad = np.full(nt * P, -1.0, dtype=np.float32)
                gat_idx[c, ti:ti + nt] = fpad.reshape(nt, P).astype(np.int16)
                dest_loc[c, ti:ti + nt] = dpad.reshape(nt, P)
                ti += nt
        assert ti == T_total

    # snake-16 idx layout per chunk, concatenated along free dim
    idx16 = np.zeros((N_CORES, P, T_total * (P // 16)), dtype=np.int16)
    for c in range(N_CORES):
        col = 0
        for (t0, nt, h) in chunks:
            nidx = nt * P
            flat = gat_idx[c, t0:t0 + nt].reshape(nidx)   # logical i = (t-t0)*128+p
            w = flat.reshape(nidx // 16, 16).T            # [16, nidx//16]
            idx16[c, :, col:col + nidx // 16] = np.tile(w, (8, 1))
            col += nidx // 16
        assert col == T_total * (P // 16)

    dest_all = np.ascontiguousarray(dest_loc.transpose(0, 2, 1))  # [C, 128, T]

    invd_b = np.empty((N_CORES, P, NPC), dtype=np.float32)
    for c in range(N_CORES):
        invd_b[c] = np.broadcast_to(invd[c * NPC:(c + 1) * NPC], (P, NPC))

    Wc = np.asarray(W_comb, dtype=np.float32)
    meta = dict(N=N, E=E, IN=IN_DIM, L=LATENT, OUT=OUT_DIM, NLAYER=NLAYER,
                NPC=NPC, NBLK=NBLK, tiles=tiles, chunks=chunks, T_total=T_total,
                has_half=has_half)
    host = dict(
        XT=XT, idx16=idx16, dest_all=dest_all, invd_b=invd_b,
        W_in=np.asarray(W_in, np.float32),
        b_in=np.asarray(b_in, np.float32).reshape(LATENT, 1),
        Wc=Wc,
        bc=np.asarray(b_comb, np.float32).reshape(NLAYER, LATENT, 1),
        W_out=np.asarray(W_out, np.float32),
        b_out_b=np.broadcast_to(np.asarray(b_out, np.float32), (P, OUT_DIM)).copy(),
    )
    return meta, host


def _build_program(meta, timing_reps=1, single_core=False):
    import concourse.bass as bass
    import concourse.bacc as bacc
    import concourse.tile as tile
    import concourse.mybir as mybir
    from concourse.masks import make_identity

    N, IN_DIM, L, OUT_DIM = meta["N"], meta["IN"], meta["L"], meta["OUT"]
    NLAYER, NPC, NBLK = meta["NLAYER"], meta["NPC"], meta["NBLK"]
    tiles, chunks, T_total = meta["tiles"], meta["chunks"], meta["T_total"]
    IW = T_total * (P // 16)
    f32 = mybir.dt.float32

    nc = bacc.Bacc("TRN2", target_bir_lowering=False, debug=False,
                   num_devices=1 if single_core else N_CORES,
                   num_swdge_queues=NQ)

    xt_in = nc.dram_tensor("xt", [IN_DIM, NPC], f32, kind="ExternalInput").ap()
    idx_in = nc.dram_tensor("idx16", [P, IW], mybir.dt.int16, kind="ExternalInput").ap()
    dest_in = nc.dram_tensor("dest", [P, T_total], f32, kind="ExternalInput").ap()
    invd_in = nc.dram_tensor("invd", [P, NPC], f32, kind="ExternalInput").ap()
    win_in = nc.dram_tensor("w_in", [IN_DIM, L], f32, kind="ExternalInput").ap()
    bin_in = nc.dram_tensor("b_in", [L, 1], f32, kind="ExternalInput").ap()
    wc_in = nc.dram_tensor("w_comb", [NLAYER, 2 * L, L], f32, kind="ExternalInput").ap()
    bc_in = nc.dram_tensor("b_comb", [NLAYER, L, 1], f32, kind="ExternalInput").ap()
    wout_in = nc.dram_tensor("w_out", [L, OUT_DIM], f32, kind="ExternalInput").ap()
    bout_in = nc.dram_tensor("b_out", [P, OUT_DIM], f32, kind="ExternalInput").ap()
    tok_in = nc.dram_tensor("tok", [P, 1], f32, kind="ExternalInput").ap()
    out_t = nc.dram_tensor("out", [NPC, OUT_DIM], f32, kind="ExternalOutput").ap()
    tok_out = nc.dram_tensor("tok_out", [P, 1], f32, kind="ExternalOutput").ap()

    NT_NODE = (NPC + P - 1) // P          # node tiles (49)
    CCH = 512                              # combine chunk width
    NCH = (NPC + CCH - 1) // CCH

    with tile.TileContext(nc) as tc:
        with (
            tc.tile_pool(name="const", bufs=1) as cpool,
            tc.tile_pool(name="hbuf", bufs=1) as hpool,
            tc.tile_pool(name="msg", bufs=1) as mpool,
            tc.tile_pool(name="xt", bufs=3) as xtpool,
            tc.tile_pool(name="g", bufs=4) as gpool,
            tc.tile_pool(name="s", bufs=4) as spool,
            tc.tile_pool(name="rows", bufs=3) as rpool,
            tc.tile_pool(name="osb", bufs=3) as opool,
            tc.tile_pool(name="pm", bufs=2, space="PSUM") as pm,       # msg blocks
            tc.tile_pool(name="pc", bufs=2, space="PSUM") as pc,       # combine
            tc.tile_pool(name="pt", bufs=2, space="PSUM") as pt,       # transpose
            tc.tile_pool(name="ph", bufs=2, space="PSUM") as ph,       # head
            tc.tile_pool(name="dram", bufs=1, space="DRAM") as dram,
        ):
            # ---- constants / metadata
            ident = cpool.tile([P, P], f32)
            make_identity(nc, ident[:])
            iota_i = cpool.tile([P, P], mybir.dt.int32)
            nc.gpsimd.iota(iota_i[:], pattern=[[1, P]], base=0, channel_multiplier=0)
            iota_f = cpool.tile([P, P], f32)
            nc.vector.tensor_copy(iota_f[:], iota_i[:])

            idx16_t = cpool.tile([P, IW], mybir.dt.int16)
            nc.sync.dma_start(out=idx16_t[:], in_=idx_in[:])
            dest_t = cpool.tile([P, T_total], f32)
            nc.sync.dma_start(out=dest_t[:], in_=dest_in[:])
            invd_t = cpool.tile([P, NPC], f32)
            nc.sync.dma_start(out=invd_t[:], in_=invd_in[:])
            win_t = cpool.tile([IN_DIM, L], f32)
            nc.sync.dma_start(out=win_t[:], in_=win_in[:])
            bin_t = cpool.tile([L, 1], f32)
            nc.sync.dma_start(out=bin_t[:], in_=bin_in[:])
            wc_t = [[cpool.tile([L, L], f32, tag=f"wc{l}{k}", name=f"wc{l}{k}")
                     for k in range(2)] for l in range(NLAYER)]
            bc_t = [cpool.tile([L, 1], f32, tag=f"bc{l}", name=f"bc{l}")
                    for l in range(NLAYER)]
            for l in range(NLAYER):
                nc.sync.dma_start(out=wc_t[l][0][:], in_=wc_in[l, 0:L, :])
                nc.sync.dma_start(out=wc_t[l][1][:], in_=wc_in[l, L:2 * L, :])
                nc.sync.dma_start(out=bc_t[l][:], in_=bc_in[l])
            wout_t = cpool.tile([L, OUT_DIM], f32)
            nc.sync.dma_start(out=wout_t[:], in_=wout_in[:])
            bout_t = cpool.tile([P, OUT_DIM], f32)
            nc.sync.dma_start(out=bout_t[:], in_=bout_in[:])
            tokt = cpool.tile([P, 1], f32, name="tokt")
            nc.sync.dma_start(out=tokt[:], in_=tok_in[:])
            nc.sync.dma_start(out=tok_out[:], in_=tokt[:])

            def rows_and_bounce(hT, bounce):
                """PE-transpose hT [L, NPC] tiles into row layout, DMA to bounce."""
                for ntile in range(NT_NODE):
                    w = min(P, NPC - ntile * P)
                    sl = slice(ntile * P, ntile * P + w)
                    tps = pt.tile([P, P], f32, tag="tp")
                    nc.tensor.transpose(out=tps[:w, :], in_=hT[:, sl], identity=ident[:])
                    rows = rpool.tile([P, P], f32, tag="rows")
                    nc.vector.tensor_copy(rows[:w, :], tps[:w, :])
                    nc.sync.dma_start(out=bounce[sl, :], in_=rows[:w, :])

            def layer_msg_combine(li, hT_prev, hT_next, tab, qoff):
                msgT = mpool.tile([P, NPC], f32, tag="msgT")
                psum_blk = {}
                has_half = meta["has_half"]
                for ci, (t0, nt, half) in enumerate(chunks):
                    g = gpool.tile([P, CHUNK_TILES * P], f32, tag="g")
                    src = tab[0:SPLIT, :] if half == 0 else tab[SPLIT:N, :]
                    nidx = nt * P
                    nc.gpsimd.dma_gather(
                        out_ap=g[:, 0:nt * P].rearrange("p (t d) -> p t d", d=P),
                        in_ap=src,
                        idxs_ap=idx16_t[:, t0 * (P // 16):(t0 + nt) * (P // 16)],
                        num_idxs=nidx, num_idxs_reg=nidx, elem_size=P,
                        queue_num=(qoff + ci) % NQ,
                    )
                    # chunked selection build: S[p, j, d] = (dest[p, t0+j] == d)
                    S = spool.tile([P, CHUNK_TILES * P], f32, tag="s")
                    d_ap = dest_t[:, t0:t0 + nt]
                    d_bc = bass.AP(d_ap.tensor, d_ap.offset,
                                   [list(d_ap.ap[0]), list(d_ap.ap[1]), [0, P]])
                    i_ap = iota_f[:]
                    i_bc = bass.AP(i_ap.tensor, i_ap.offset,
                                   [list(i_ap.ap[0]), [0, nt], list(i_ap.ap[1])])
                    nc.vector.tensor_tensor(
                        out=S[:, 0:nt * P].rearrange("p (t d) -> p t d", d=P),
                        in0=d_bc, in1=i_bc, op=mybir.AluOpType.is_equal)
                    for j in range(nt):
                        t = t0 + j
                        b, hh, first, last = tiles[t]
                        if first:
                            psum_blk[b] = pm.tile([P, P], f32, tag="pmblk",
                                                  name=f"pmblk{li}_{b}_{hh}")
                        nc.tensor.matmul(
                            out=psum_blk[b][:],
                            lhsT=g[:, j * P:(j + 1) * P],
                            rhs=S[:, j * P:(j + 1) * P],
                            start=first, stop=last,
                        )
                        if last:
                            wb = min(P, NPC - b * P)
                            bsl = slice(b * P, b * P + wb)
                            if hh == 0 or not has_half[b][0]:
                                nc.vector.tensor_copy(msgT[:, bsl],
                                                      psum_blk[b][:, :wb])
                            else:
                                nc.vector.tensor_tensor(
                                    out=msgT[:, bsl], in0=psum_blk[b][:, :wb],
                                    in1=msgT[:, bsl], op=mybir.AluOpType.add)
                            del psum_blk[b]
                # blocks with no tiles at all: zero msg
                for b in range(NBLK):
                    if not (has_half[b][0] or has_half[b][1]):
                        wb = min(P, NPC - b * P)
                        nc.vector.memset(msgT[:, b * P:b * P + wb], 0.0)
                # msgT *= invdeg (chunked), then combine
                for j in range(NCH):
                    w = min(CCH, NPC - j * CCH)
                    sl = slice(j * CCH, j * CCH + w)
                    nc.vector.tensor_tensor(out=msgT[:, sl], in0=msgT[:, sl],
                                            in1=invd_t[:, sl],
                                            op=mybir.AluOpType.mult)
                for j in range(NCH):
                    w = min(CCH, NPC - j * CCH)
                    sl = slice(j * CCH, j * CCH + w)
                    psum = pc.tile([P, CCH], f32, tag="pcomb")
                    nc.tensor.matmul(out=psum[:, :w], lhsT=wc_t[li][0][:],
                                     rhs=hT_prev[:, sl], start=True, stop=False)
                    nc.tensor.matmul(out=psum[:, :w], lhsT=wc_t[li][1][:],
                                     rhs=msgT[:, sl], start=False, stop=True)
                    nc.scalar.activation(hT_next[:, sl], psum[:, :w],
                                         mybir.ActivationFunctionType.Relu,
                                         bias=bc_t[li][:])

            for rep in range(timing_reps):
                qoff = 0
                # ---- h0 = relu(W_in^T @ X^T + b_in), transposed layout
                hT = [hpool.tile([P, NPC], f32, tag=f"hT{l % 2}", name=f"hT{l}_{rep}")
                      for l in range(NLAYER + 1)]
                for j in range(NCH):
                    w = min(CCH, NPC - j * CCH)
                    sl = slice(j * CCH, j * CCH + w)
                    xt_c = xtpool.tile([IN_DIM, CCH], f32, tag="xt")
                    nc.sync.dma_start(out=xt_c[:, :w], in_=xt_in[:, sl])
                    psum = pc.tile([P, CCH], f32, tag="pcomb")
                    nc.tensor.matmul(out=psum[:, :w], lhsT=win_t[:],
                                     rhs=xt_c[:, :w], start=True, stop=True)
                    nc.scalar.activation(hT[0][:, sl], psum[:, :w],
                                         mybir.ActivationFunctionType.Relu,
                                         bias=bin_t[:])

                # ---- per layer: publish rows, allgather, message+combine
                for li in range(NLAYER):
                    bounce = dram.tile([NPC, P], f32, tag=f"bounce{li}_{rep}")
                    tab = dram.tile([N, P], f32, tag=f"tab{li}_{rep}",
                                    addr_space="Local" if single_core else "Shared")
                    rows_and_bounce(hT[li][:], bounce)
                    if single_core:
                        nc.sync.dma_start(out=tab[0:NPC, :], in_=bounce[:])
                    else:
                        nc.gpsimd.collective_compute(
                            "AllGather", mybir.AluOpType.bypass,
                            ins=[bounce[:]], outs=[tab[:]],
                            replica_groups=[list(range(N_CORES))],
                        )
                    layer_msg_combine(li, hT[li][:], hT[li + 1][:], tab, qoff)
                    qoff += len(chunks)

                # ---- output head: out rows = hT_last^T @ W_out + b_out
                for ntile in range(NT_NODE):
                    w = min(P, NPC - ntile * P)
                    sl = slice(ntile * P, ntile * P + w)
                    psum = ph.tile([P, OUT_DIM], f32, tag="phead")
                    nc.tensor.matmul(out=psum[:w, :], lhsT=hT[NLAYER][:, sl],
                                     rhs=wout_t[:], start=True, stop=True)
                    o = opool.tile([P, OUT_DIM], f32, tag="o")
                    nc.vector.tensor_tensor(out=o[:w, :], in0=psum[:w, :],
                                            in1=bout_t[:w, :],
                                            op=mybir.AluOpType.add)
                    nc.sync.dma_start(out=out_t[sl, :], in_=o[:w, :])

    nc.compile()
    return nc


def _in_maps(meta, host):
    maps = []
    NPC = meta["NPC"]
    for c in range(N_CORES):
        maps.append({
            "xt": np.ascontiguousarray(host["XT"][:, c * NPC:(c + 1) * NPC]),
            "idx16": host["idx16"][c],
            "dest": host["dest_all"][c],
            "invd": host["invd_b"][c],
            "w_in": host["W_in"],
            "b_in": host["b_in"],
            "w_comb": host["Wc"],
            "b_comb": host["bc"],
            "w_out": host["W_out"],
            "b_out": host["b_out_b"],
            "tok": np.zeros((P, 1), np.float32),
        })
    return maps


def kernel(nodes_feats, adj_list, W_in, b_in, W_comb, b_comb, W_out, b_out):
    from concourse.bass_utils import run_bass_kernel_spmd

    meta, host = _host_prep(nodes_feats, adj_list, W_in, b_in, W_comb, b_comb,
                            W_out, b_out)
    nc = _build_program(meta)
    res = run_bass_kernel_spmd(nc, _in_maps(meta, host),
                               core_ids=list(range(N_CORES)))
    out = np.concatenate([res.results[c]["out"] for c in range(N_CORES)], axis=0)
    return out.astype(np.float32)


# revision 21
# speedup vs baseline: 1.2959x; 1.0742x over previous
"""GNN message-passing kernel for 8 Trainium2 NeuronCores.

Strategy: 1-D node partition (6250 nodes/core). Each layer:
  - node features h kept in transposed SBUF layout hT [128 latent, nodes]
  - h rows are PE-transposed out and AllGathered into a full [50000, 128]
    DRAM table per layer
  - edges are owned by the destination core, sorted by (dest block, src half);
    h[fro] is fetched with dma_gather (int16 idx -> table split in two halves)
  - segment-sum via PE matmul: msgT_block += M_tile^T @ S_tile where S is the
    per-tile one-hot dest selection matrix built on DVE (iota == dest)
  - combine: hT_next = relu(W1^T hT + W2^T (msgT * invdeg) + b)
"""

import os
import numpy as np

N_CORES = 8
P = 128
CHUNK_TILES = 8          # tiles per dma_gather instruction (<=1024 idx, ring cap)
NQ = 4                   # SWDGE queues for gather round-robin
SPLIT = 32768            # int16-addressable table half boundary
BF16_TABLE = True        # gather tables / S / allgather payload in bf16


def _host_prep(nodes_feats, adj_list, W_in, b_in, W_comb, b_comb, W_out, b_out):
    N, IN_DIM = nodes_feats.shape
    E = adj_list.shape[0]
    LATENT = W_in.shape[1]
    OUT_DIM = W_out.shape[1]
    NLAYER = W_comb.shape[0]
    NPC = N // N_CORES
    NBLK = (NPC + P - 1) // P

    fro = np.asarray(adj_list[:, 0], dtype=np.int64)
    to = np.asarray(adj_list[:, 1], dtype=np.int64)
    deg = np.bincount(to, minlength=N).astype(np.float32)
    invd = 1.0 / np.maximum(deg, 1.0)

    XT = np.ascontiguousarray(np.asarray(nodes_feats, dtype=np.float32).T)  # [IN, N]

    # per (core, block, half) edge lists
    per = [[[None, None] for _ in range(NBLK)] for _ in range(N_CORES)]
    counts = np.zeros((N_CORES, NBLK, 2), dtype=np.int64)
    for c in range(N_CORES):
        sel = (to >= c * NPC) & (to < (c + 1) * NPC)
        f_c = fro[sel]
        d_c = to[sel] - c * NPC
        blk = d_c // P
        half = (f_c >= SPLIT).astype(np.int64)
        order = np.lexsort((half, blk))
        f_c, d_c, blk, half = f_c[order], d_c[order], blk[order], half[order]
        # boundaries per (blk, half)
        key = blk * 2 + half
        for k in np.unique(key):
            m = key == k
            b, h = int(k) // 2, int(k) % 2
            per[c][b][h] = (f_c[m], d_c[m] - b * P)
            counts[c, b, h] = int(m.sum())

    # static tiles-per-(block,half): max over cores
    tp = np.zeros((NBLK, 2), dtype=np.int64)
    for b in range(NBLK):
        for h in range(2):
            mx = counts[:, b, h].max()
            tp[b, h] = -(-mx // P)  # ceil; 0 if no core has edges here

    # static tile list: phase A (all A-tiles, blocks ascending), then phase B.
    # Each (phase, block) run is a separate PSUM accumulation group; phase-B
    # copyout adds into msgT (phase-A copies).
    tiles = []          # (block, half, first_of_run, last_of_run)
    for h in range(2):
        for b in range(NBLK):
            nt = int(tp[b, h])
            for j in range(nt):
                tiles.append((b, h, j == 0, j == nt - 1))
    T_total = len(tiles)
    has_half = [[tp[b, 0] > 0, tp[b, 1] > 0] for b in range(NBLK)]

    # chunks: same-half runs (span blocks), split to <= CHUNK_TILES
    chunks = []         # (t0, ntiles, half)
    t = 0
    while t < T_total:
        h = tiles[t][1]
        r = t
        while r < T_total and tiles[r][1] == h and r - t < CHUNK_TILES:
            r += 1
        chunks.append((t, r - t, h))
        t = r

    # per-core slot arrays
    gat_idx = np.zeros((N_CORES, T_total, P), dtype=np.int16)
    dest_loc = np.full((N_CORES, T_total, P), -1.0, dtype=np.float32)
    for c in range(N_CORES):
        ti = 0
        for h in range(2):
            for b in range(NBLK):
                nt = int(tp[b, h])
                if nt == 0:
                    continue
                lst = per[c][b][h]
                if lst is not None:
                    f_l, d_l = lst
                    n = len(f_l)
                    fpad = np.zeros(nt * P, dtype=np.int64)
                    dpad = np.full(nt * P, -1.0, dtype=np.float32)
                    fpad[:n] = f_l - (SPLIT if h else 0)
                    dpad[:n] = d_l
                else:
                    fpad = np.zeros(nt * P, dtype=np.int64)
                    dpad = np.full(nt * P, -1.0, dtype=np.float32)
                gat_idx[c, ti:ti + nt] = fpad.reshape(nt, P).astype(np.int16)
                dest_loc[c, ti:ti + nt] = dpad.reshape(nt, P)
                ti += nt
        assert ti == T_total

    # snake-16 idx layout per chunk, concatenated along free dim
    idx16 = np.zeros((N_CORES, P, T_total * (P // 16)), dtype=np.int16)
    for c in range(N_CORES):
        col = 0
        for (t0, nt, h) in chunks:
            nidx = nt * P
            flat = gat_idx[c, t0:t0 + nt].reshape(nidx)   # logical i = (t-t0)*128+p
            w = flat.reshape(nidx // 16, 16).T            # [16, nidx//16]
            idx16[c, :, col:col + nidx // 16] = np.tile(w, (8, 1))
            col += nidx // 16
        assert col == T_total * (P // 16)

    dest_all = np.ascontiguousarray(dest_loc.transpose(0, 2, 1))  # [C, 128, T]
    if BF16_TABLE:
        import ml_dtypes
        dest_all = dest_all.astype(ml_dtypes.bfloat16)

    invd_b = np.empty((N_CORES, P, NPC), dtype=np.float32)
    for c in range(N_CORES):
        invd_b[c] = np.broadcast_to(invd[c * NPC:(c + 1) * NPC], (P, NPC))

    Wc = np.asarray(W_comb, dtype=np.float32)
    meta = dict(N=N, E=E, IN=IN_DIM, L=LATENT, OUT=OUT_DIM, NLAYER=NLAYER,
                NPC=NPC, NBLK=NBLK, tiles=tiles, chunks=chunks, T_total=T_total,
                has_half=has_half)
    host = dict(
        XT=XT, idx16=idx16, dest_all=dest_all, invd_b=invd_b,
        W_in=np.asarray(W_in, np.float32),
        b_in=np.asarray(b_in, np.float32).reshape(LATENT, 1),
        Wc=Wc,
        bc=np.asarray(b_comb, np.float32).reshape(NLAYER, LATENT, 1),
        W_out=np.asarray(W_out, np.float32),
        b_out_b=np.broadcast_to(np.asarray(b_out, np.float32), (P, OUT_DIM)).copy(),
    )
    return meta, host


def _build_program(meta, timing_reps=1, single_core=False, no_collective=False):
    import concourse.bass as bass
    import concourse.bacc as bacc
    import concourse.tile as tile
    import concourse.mybir as mybir
    from concourse.masks import make_identity

    N, IN_DIM, L, OUT_DIM = meta["N"], meta["IN"], meta["L"], meta["OUT"]
    NLAYER, NPC, NBLK = meta["NLAYER"], meta["NPC"], meta["NBLK"]
    tiles, chunks, T_total = meta["tiles"], meta["chunks"], meta["T_total"]
    IW = T_total * (P // 16)
    f32 = mybir.dt.float32
    tdt = mybir.dt.bfloat16 if BF16_TABLE else f32

    nc = bacc.Bacc("TRN2", target_bir_lowering=False, debug=False,
                   num_devices=1 if single_core else N_CORES,
                   num_swdge_queues=NQ)

    xt_in = nc.dram_tensor("xt", [IN_DIM, NPC], f32, kind="ExternalInput").ap()
    idx_in = nc.dram_tensor("idx16", [P, IW], mybir.dt.int16, kind="ExternalInput").ap()
    dest_in = nc.dram_tensor("dest", [P, T_total], tdt, kind="ExternalInput").ap()
    invd_in = nc.dram_tensor("invd", [P, NPC], f32, kind="ExternalInput").ap()
    win_in = nc.dram_tensor("w_in", [IN_DIM, L], f32, kind="ExternalInput").ap()
    bin_in = nc.dram_tensor("b_in", [L, 1], f32, kind="ExternalInput").ap()
    wc_in = nc.dram_tensor("w_comb", [NLAYER, 2 * L, L], f32, kind="ExternalInput").ap()
    bc_in = nc.dram_tensor("b_comb", [NLAYER, L, 1], f32, kind="ExternalInput").ap()
    wout_in = nc.dram_tensor("w_out", [L, OUT_DIM], f32, kind="ExternalInput").ap()
    bout_in = nc.dram_tensor("b_out", [P, OUT_DIM], f32, kind="ExternalInput").ap()
    tok_in = nc.dram_tensor("tok", [P, 1], f32, kind="ExternalInput").ap()
    out_t = nc.dram_tensor("out", [NPC, OUT_DIM], f32, kind="ExternalOutput").ap()
    tok_out = nc.dram_tensor("tok_out", [P, 1], f32, kind="ExternalOutput").ap()

    NT_NODE = (NPC + P - 1) // P          # node tiles (49)
    CCH = 512                              # combine chunk width
    NCH = (NPC + CCH - 1) // CCH

    with tile.TileContext(nc) as tc:
        with (
            tc.tile_pool(name="const", bufs=1) as cpool,
            tc.tile_pool(name="hbuf", bufs=1) as hpool,
            tc.tile_pool(name="msg", bufs=1) as mpool,
            tc.tile_pool(name="xt", bufs=3) as xtpool,
            tc.tile_pool(name="g", bufs=4) as gpool,
            tc.tile_pool(name="s", bufs=4) as spool,
            tc.tile_pool(name="rows", bufs=3) as rpool,
            tc.tile_pool(name="osb", bufs=3) as opool,
            tc.tile_pool(name="pm", bufs=2, space="PSUM") as pm,       # msg blocks
            tc.tile_pool(name="pc", bufs=2, space="PSUM") as pc,       # combine
            tc.tile_pool(name="pt", bufs=2, space="PSUM") as pt,       # transpose
            tc.tile_pool(name="ph", bufs=2, space="PSUM") as ph,       # head
            tc.tile_pool(name="dram", bufs=1, space="DRAM") as dram,
        ):
            # ---- constants / metadata
            ident = cpool.tile([P, P], f32)
            make_identity(nc, ident[:])
            iota_i = cpool.tile([P, P], mybir.dt.int32)
            nc.gpsimd.iota(iota_i[:], pattern=[[1, P]], base=0, channel_multiplier=0)
            iota_f = cpool.tile([P, P], tdt)
            nc.vector.tensor_copy(iota_f[:], iota_i[:])

            idx16_t = cpool.tile([P, IW], mybir.dt.int16)
            nc.sync.dma_start(out=idx16_t[:], in_=idx_in[:])
            dest_t = cpool.tile([P, T_total], tdt)
            nc.sync.dma_start(out=dest_t[:], in_=dest_in[:])
            invd_t = cpool.tile([P, NPC], f32)
            nc.sync.dma_start(out=invd_t[:], in_=invd_in[:])
            win_t = cpool.tile([IN_DIM, L], f32)
            nc.sync.dma_start(out=win_t[:], in_=win_in[:])
            bin_t = cpool.tile([L, 1], f32)
            nc.sync.dma_start(out=bin_t[:], in_=bin_in[:])
            wc_t = [[cpool.tile([L, L], f32, tag=f"wc{l}{k}", name=f"wc{l}{k}")
                     for k in range(2)] for l in range(NLAYER)]
            bc_t = [cpool.tile([L, 1], f32, tag=f"bc{l}", name=f"bc{l}")
                    for l in range(NLAYER)]
            for l in range(NLAYER):
                nc.sync.dma_start(out=wc_t[l][0][:], in_=wc_in[l, 0:L, :])
                nc.sync.dma_start(out=wc_t[l][1][:], in_=wc_in[l, L:2 * L, :])
                nc.sync.dma_start(out=bc_t[l][:], in_=bc_in[l])
            wout_t = cpool.tile([L, OUT_DIM], f32)
            nc.sync.dma_start(out=wout_t[:], in_=wout_in[:])
            bout_t = cpool.tile([P, OUT_DIM], f32)
            nc.sync.dma_start(out=bout_t[:], in_=bout_in[:])
            tokt = cpool.tile([P, 1], f32, name="tokt")
            nc.sync.dma_start(out=tokt[:], in_=tok_in[:])
            nc.sync.dma_start(out=tok_out[:], in_=tokt[:])

            def rows_and_bounce(hT, bounce):
                """PE-transpose hT [L, NPC] tiles into row layout, DMA to bounce."""
                for ntile in range(NT_NODE):
                    w = min(P, NPC - ntile * P)
                    sl = slice(ntile * P, ntile * P + w)
                    tps = pt.tile([P, P], f32, tag="tp")
                    nc.tensor.transpose(out=tps[:w, :], in_=hT[:, sl], identity=ident[:])
                    rows = rpool.tile([P, P], tdt, tag="rows")
                    nc.vector.tensor_copy(rows[:w, :], tps[:w, :])
                    nc.sync.dma_start(out=bounce[sl, :], in_=rows[:w, :])

            def layer_msg_combine(li, hT_prev, hT_next, tab, qoff):
                msgT = mpool.tile([P, NPC], f32, tag="msgT")
                psum_blk = {}
                has_half = meta["has_half"]
                for ci, (t0, nt, half) in enumerate(chunks):
                    g = gpool.tile([P, CHUNK_TILES * P], tdt, tag="g")
                    src = tab[0:SPLIT, :] if half == 0 else tab[SPLIT:N, :]
                    nidx = nt * P
                    nc.gpsimd.dma_gather(
                        out_ap=g[:, 0:nt * P].rearrange("p (t d) -> p t d", d=P),
                        in_ap=src,
                        idxs_ap=idx16_t[:, t0 * (P // 16):(t0 + nt) * (P // 16)],
                        num_idxs=nidx, num_idxs_reg=nidx, elem_size=P,
                        queue_num=(qoff + ci) % NQ,
                    )
                    # chunked selection build: S[p, j, d] = (dest[p, t0+j] == d)
                    S = spool.tile([P, CHUNK_TILES * P], tdt, tag="s")
                    d_ap = dest_t[:, t0:t0 + nt]
                    d_bc = bass.AP(d_ap.tensor, d_ap.offset,
                                   [list(d_ap.ap[0]), list(d_ap.ap[1]), [0, P]])
                    i_ap = iota_f[:]
                    i_bc = bass.AP(i_ap.tensor, i_ap.offset,
                                   [list(i_ap.ap[0]), [0, nt], list(i_ap.ap[1])])
                    nc.vector.tensor_tensor(
                        out=S[:, 0:nt * P].rearrange("p (t d) -> p t d", d=P),
                        in0=d_bc, in1=i_bc, op=mybir.AluOpType.is_equal)
                    for j in range(nt):
                        t = t0 + j
                        b, hh, first, last = tiles[t]
                        if first:
                            psum_blk[b] = pm.tile([P, P], f32, tag="pmblk",
                                                  name=f"pmblk{li}_{b}_{hh}")
                        nc.tensor.matmul(
                            out=psum_blk[b][:],
                            lhsT=g[:, j * P:(j + 1) * P],
                            rhs=S[:, j * P:(j + 1) * P],
                            start=first, stop=last,
                        )
                        if last:
                            wb = min(P, NPC - b * P)
                            bsl = slice(b * P, b * P + wb)
                            if hh == 0 or not has_half[b][0]:
                                nc.vector.tensor_copy(msgT[:, bsl],
                                                      psum_blk[b][:, :wb])
                            else:
                                nc.vector.tensor_tensor(
                                    out=msgT[:, bsl], in0=psum_blk[b][:, :wb],
                                    in1=msgT[:, bsl], op=mybir.AluOpType.add)
                            del psum_blk[b]
                # blocks with no tiles at all: zero msg
                for b in range(NBLK):
                    if not (has_half[b][0] or has_half[b][1]):
                        wb = min(P, NPC - b * P)
                        nc.vector.memset(msgT[:, b * P:b * P + wb], 0.0)
                # msgT *= invdeg (chunked), then combine
                for j in range(NCH):
                    w = min(CCH, NPC - j * CCH)
                    sl = slice(j * CCH, j * CCH + w)
                    nc.vector.tensor_tensor(out=msgT[:, sl], in0=msgT[:, sl],
                                            in1=invd_t[:, sl],
                                            op=mybir.AluOpType.mult)
                for j in range(NCH):
                    w = min(CCH, NPC - j * CCH)
                    sl = slice(j * CCH, j * CCH + w)
                    psum = pc.tile([P, CCH], f32, tag="pcomb")
                    nc.tensor.matmul(out=psum[:, :w], lhsT=wc_t[li][0][:],
                                     rhs=hT_prev[:, sl], start=True, stop=False)
                    nc.tensor.matmul(out=psum[:, :w], lhsT=wc_t[li][1][:],
                                     rhs=msgT[:, sl], start=False, stop=True)
                    nc.scalar.activation(hT_next[:, sl], psum[:, :w],
                                         mybir.ActivationFunctionType.Relu,
                                         bias=bc_t[li][:])

            for rep in range(timing_reps):
                qoff = 0
                # ---- h0 = relu(W_in^T @ X^T + b_in), transposed layout
                hT = [hpool.tile([P, NPC], f32, tag=f"hT{l % 2}", name=f"hT{l}_{rep}")
                      for l in range(NLAYER + 1)]
                for j in range(NCH):
                    w = min(CCH, NPC - j * CCH)
                    sl = slice(j * CCH, j * CCH + w)
                    xt_c = xtpool.tile([IN_DIM, CCH], f32, tag="xt")
                    nc.sync.dma_start(out=xt_c[:, :w], in_=xt_in[:, sl])
                    psum = pc.tile([P, CCH], f32, tag="pcomb")
                    nc.tensor.matmul(out=psum[:, :w], lhsT=win_t[:],
                                     rhs=xt_c[:, :w], start=True, stop=True)
                    nc.scalar.activation(hT[0][:, sl], psum[:, :w],
                                         mybir.ActivationFunctionType.Relu,
                                         bias=bin_t[:])

                # ---- per layer: publish rows, allgather, message+combine
                for li in range(NLAYER):
                    bounce = dram.tile([NPC, P], tdt, tag=f"bounce{li}_{rep}")
                    tab = dram.tile([N, P], tdt, tag=f"tab{li}_{rep}",
                                    addr_space="Local" if single_core else "Shared")
                    rows_and_bounce(hT[li][:], bounce)
                    if single_core or no_collective:
                        nc.sync.dma_start(out=tab[0:NPC, :], in_=bounce[:])
                    else:
                        nc.gpsimd.collective_compute(
                            "AllGather", mybir.AluOpType.bypass,
                            ins=[bounce[:]], outs=[tab[:]],
                            replica_groups=[list(range(N_CORES))],
                        )
                    layer_msg_combine(li, hT[li][:], hT[li + 1][:], tab, qoff)
                    qoff += len(chunks)

                # ---- output head: out rows = hT_last^T @ W_out + b_out
                for ntile in range(NT_NODE):
                    w = min(P, NPC - ntile * P)
                    sl = slice(ntile * P, ntile * P + w)
                    psum = ph.tile([P, OUT_DIM], f32, tag="phead")
                    nc.tensor.matmul(out=psum[:w, :], lhsT=hT[NLAYER][:, sl],
                                     rhs=wout_t[:], start=True, stop=True)
                    o = opool.tile([P, OUT_DIM], f32, tag="o")
                    nc.vector.tensor_tensor(out=o[:w, :], in0=psum[:w, :],
                                            in1=bout_t[:w, :],
                                            op=mybir.AluOpType.add)
                    nc.sync.dma_start(out=out_t[sl, :], in_=o[:w, :])

    nc.compile()
    return nc


def _in_maps(meta, host):
    maps = []
    NPC = meta["NPC"]
    for c in range(N_CORES):
        maps.append({
            "xt": np.ascontiguousarray(host["XT"][:, c * NPC:(c + 1) * NPC]),
            "idx16": host["idx16"][c],
            "dest": host["dest_all"][c],
            "invd": host["invd_b"][c],
            "w_in": host["W_in"],
            "b_in": host["b_in"],
            "w_comb": host["Wc"],
            "b_comb": host["bc"],
            "w_out": host["W_out"],
            "b_out": host["b_out_b"],
            "tok": np.zeros((P, 1), np.float32),
        })
    return maps


def kernel(nodes_feats, adj_list, W_in, b_in, W_comb, b_comb, W_out, b_out):
    from concourse.bass_utils import run_bass_kernel_spmd

    meta, host = _host_prep(nodes_feats, adj_list, W_in, b_in, W_comb, b_comb,
                            W_out, b_out)
    nc = _build_program(meta)
    res = run_bass_kernel_spmd(nc, _in_maps(meta, host),
                               core_ids=list(range(N_CORES)))
    out = np.concatenate([res.results[c]["out"] for c in range(N_CORES)], axis=0)
    return out.astype(np.float32)
